# revision 5
# baseline (speedup 1.0000x reference)
"""Trainium2 Bass kernel for the MU-MISO channel problem.

Math: the reference collapses algebraically to a 4x4 channel mix over the
huge [B, C] axis plus scaled noise:

    out[u, b, c] = sum_v M'[u, v] * x[v, b, c] + s'[u] * noise[u, b, c]

where  A[u, v]  = sqrt(P[v]) * sum_n H[n, u] * W[n, v]
       amp[u]   = A[u, u]
       M'       = A / amp[:, None]
       s'       = stddev / amp

M'/s' are tiny (4x4 / 4) and computed on host from W/H/P/stddev; the
O(U*B*C) streaming work runs on 8 NeuronCores, data-parallel over Batch.

Per-core layout: the per-core shard x_s[u, :] (N = 16*49152 elems) is viewed
as [U=4, Q=32, NSUP, F]; SBUF tiles are [128, F] with partition p = u*32+q.
The 4-way mix across u becomes a single 128x128 stationary matmul with
S = kron(M'.T, I_32) (block-diagonal per q), so the VectorEngine only does
one fused op per element:  out = (noise * s_pp) + psum.
"""

import sys

for _p in ("/opt/trn_rl_repo",):
    if _p not in sys.path:
        sys.path.insert(0, _p)

import numpy as np

import concourse.bass as bass
import concourse.tile as tile
from concourse import bacc, mybir
from concourse import bass_utils

# Problem shapes (hardcoded per contract)
U, NT, BATCH, CWH = 4, 8, 128, 49152
NCORES = 8
BL = BATCH // NCORES            # 16 batches per core
N = BL * CWH                    # 786432 elems per (core, u)
Q = 32                          # chunks per u -> partition p = u*32 + q
NSUP = 4                        # super-tiles (outer loop)
F = N // (Q * NSUP)             # 4096 free elems per partition per super-tile
T = 512                         # matmul free dim (one PSUM bank)
JS = F // T                     # 8 matmuls per super-tile
FP32 = mybir.dt.float32

_CACHE = {}


def _build_program():
    """Build + compile the per-core Bass program (same program on all cores)."""
    nc = bacc.Bacc(
        "TRN2",
        target_bir_lowering=False,
        debug=False,
        enable_asserts=True,
        num_devices=NCORES,
    )
    x_d = nc.dram_tensor("x_s", [U, Q, NSUP, F], FP32, kind="ExternalInput")
    n_d = nc.dram_tensor("n_s", [U, Q, NSUP, F], FP32, kind="ExternalInput")
    S_d = nc.dram_tensor("S_mat", [128, 128], FP32, kind="ExternalInput")
    s_d = nc.dram_tensor("s_pp", [128, 1], FP32, kind="ExternalInput")
    o_d = nc.dram_tensor("out_s", [U, Q, NSUP, F], FP32, kind="ExternalOutput")

    with tile.TileContext(nc) as tc:
        with (
            tc.tile_pool(name="const", bufs=1) as cpool,
            tc.tile_pool(name="io", bufs=2) as iopool,
            tc.tile_pool(name="psum", bufs=8, space="PSUM") as pspool,
        ):
            S_t = cpool.tile([128, 128], FP32)
            nc.sync.dma_start(S_t[:], S_d[:, :])
            s_t = cpool.tile([128, 1], FP32)
            nc.sync.dma_start(s_t[:], s_d[:, :])

            for st in range(NSUP):
                x_t = iopool.tile([128, F], FP32, tag="x")
                nc.sync.dma_start(x_t[:], x_d[:, :, st, :])
                n_t = iopool.tile([128, F], FP32, tag="n")
                nc.sync.dma_start(n_t[:], n_d[:, :, st, :])
                o_t = iopool.tile([128, F], FP32, tag="o")
                for k in range(JS):
                    ps = pspool.tile([128, T], FP32)
                    nc.tensor.matmul(
                        ps[:],
                        S_t[:],
                        x_t[:, k * T : (k + 1) * T],
                        start=True,
                        stop=True,
                    )
                    nc.vector.scalar_tensor_tensor(
                        out=o_t[:, k * T : (k + 1) * T],
                        in0=n_t[:, k * T : (k + 1) * T],
                        scalar=s_t[:, :],
                        in1=ps[:],
                        op0=mybir.AluOpType.mult,
                        op1=mybir.AluOpType.add,
                    )
                nc.sync.dma_start(o_d[:, :, st, :], o_t[:])

    nc.compile()
    return nc


def _get_program():
    if "nc" not in _CACHE:
        _CACHE["nc"] = _build_program()
    return _CACHE["nc"]


def _host_scalars(W, H, P, stddev):
    """M' (4x4 mix), s' (noise scale) -> S_mat [128,128], s_pp [128,1] f32."""
    W64 = np.asarray(W, np.float64)
    H64 = np.asarray(H, np.float64)
    P64 = np.asarray(P, np.float64)
    sd64 = np.asarray(stddev, np.float64)
    sqrtP = np.sqrt(P64)
    A = H64.T @ (W64 * sqrtP[None, :])  # A[u,v] = sum_n H[n,u] W[n,v] sqrtP[v]
    amp = np.diag(A).copy()
    Mp = A / amp[:, None]
    sp = sd64 / amp
    S_mat = np.kron(Mp.T, np.eye(Q, dtype=np.float64)).astype(np.float32)
    s_pp = np.repeat(sp, Q).astype(np.float32).reshape(128, 1)
    return np.ascontiguousarray(S_mat), s_pp


def make_in_maps(x, W, H, P, stddev, noise):
    S_mat, s_pp = _host_scalars(W, H, P, stddev)
    x = np.asarray(x, np.float32)
    noise = np.asarray(noise, np.float32)
    in_maps = []
    for c in range(NCORES):
        xs = np.ascontiguousarray(x[:, c * BL : (c + 1) * BL, :]).reshape(
            U, Q, NSUP, F
        )
        ns = np.ascontiguousarray(noise[:, c * BL : (c + 1) * BL, :]).reshape(
            U, Q, NSUP, F
        )
        in_maps.append({"x_s": xs, "n_s": ns, "S_mat": S_mat, "s_pp": s_pp})
    return in_maps


def gather_output(results):
    out = np.empty((U, BATCH, CWH), np.float32)
    for c in range(NCORES):
        out[:, c * BL : (c + 1) * BL, :] = results[c]["out_s"].reshape(U, BL, CWH)
    return out


def run_on_hw(x, W, H, P, stddev, noise, **run_kwargs):
    nc = _get_program()
    in_maps = make_in_maps(x, W, H, P, stddev, noise)
    res = bass_utils.run_bass_kernel_spmd(
        nc, in_maps, core_ids=list(range(NCORES)), **run_kwargs
    )
    return res


def kernel(x, W, H, P, stddev, noise):
    res = run_on_hw(x, W, H, P, stddev, noise)
    return gather_output(res.results)


# revision 6
# speedup vs baseline: 1.0979x; 1.0979x over previous
"""Trainium2 Bass kernel for the MU-MISO channel problem.

Math: the reference collapses algebraically to a 4x4 channel mix over the
huge [B, C] axis plus scaled noise:

    out[u, b, c] = sum_v M'[u, v] * x[v, b, c] + s'[u] * noise[u, b, c]

where  A[u, v]  = sqrt(P[v]) * sum_n H[n, u] * W[n, v]
       amp[u]   = A[u, u]
       M'       = A / amp[:, None]
       s'       = stddev / amp

M'/s' are tiny (4x4 / 4) and computed on host from W/H/P/stddev; the
O(U*B*C) streaming work runs on 8 NeuronCores, data-parallel over Batch.

Per-core layout: the per-core shard x_s[u, :] (N = 16*49152 elems) is viewed
as [U=4, Q=32, NSUP, F]; SBUF tiles are [128, F] with partition p = u*32+q.
The 4-way mix across u becomes a single 128x128 stationary matmul with
S = kron(M'.T, I_32) (block-diagonal per q), so the VectorEngine only does
one fused op per element:  out = (noise * s_pp) + psum.
"""

import sys

for _p in ("/opt/trn_rl_repo",):
    if _p not in sys.path:
        sys.path.insert(0, _p)

import numpy as np

import concourse.bass as bass
import concourse.tile as tile
from concourse import bacc, mybir
from concourse import bass_utils

# Problem shapes (hardcoded per contract)
U, NT, BATCH, CWH = 4, 8, 128, 49152
NCORES = 8
BL = BATCH // NCORES            # 16 batches per core
N = BL * CWH                    # 786432 elems per (core, u)
Q = 32                          # chunks per u -> partition p = u*32 + q
NSUP = 6                        # super-tiles (outer loop)
F = N // (Q * NSUP)             # 4096 free elems per partition per super-tile
T = 512                         # matmul free dim (one PSUM bank)
JS = F // T                     # 8 matmuls per super-tile
FP32 = mybir.dt.float32

_CACHE = {}


def _build_program():
    """Build + compile the per-core Bass program (same program on all cores)."""
    nc = bacc.Bacc(
        "TRN2",
        target_bir_lowering=False,
        debug=False,
        enable_asserts=True,
        num_devices=NCORES,
    )
    x_d = nc.dram_tensor("x_s", [U, Q, NSUP, F], FP32, kind="ExternalInput")
    n_d = nc.dram_tensor("n_s", [U, Q, NSUP, F], FP32, kind="ExternalInput")
    S_d = nc.dram_tensor("S_mat", [128, 128], FP32, kind="ExternalInput")
    s_d = nc.dram_tensor("s_pp", [128, 1], FP32, kind="ExternalInput")
    o_d = nc.dram_tensor("out_s", [U, Q, NSUP, F], FP32, kind="ExternalOutput")

    with tile.TileContext(nc) as tc:
        with (
            tc.tile_pool(name="const", bufs=1) as cpool,
            tc.tile_pool(name="io", bufs=3) as iopool,
            tc.tile_pool(name="psum", bufs=8, space="PSUM") as pspool,
        ):
            S_t = cpool.tile([128, 128], FP32)
            nc.sync.dma_start(S_t[:], S_d[:, :])
            s_t = cpool.tile([128, 1], FP32)
            nc.sync.dma_start(s_t[:], s_d[:, :])

            for st in range(NSUP):
                x_t = iopool.tile([128, F], FP32, tag="x")
                nc.sync.dma_start(x_t[:], x_d[:, :, st, :])
                n_t = iopool.tile([128, F], FP32, tag="n")
                nc.sync.dma_start(n_t[:], n_d[:, :, st, :])
                o_t = iopool.tile([128, F], FP32, tag="o")
                for k in range(JS):
                    ps = pspool.tile([128, T], FP32)
                    nc.tensor.matmul(
                        ps[:],
                        S_t[:],
                        x_t[:, k * T : (k + 1) * T],
                        start=True,
                        stop=True,
                    )
                    nc.vector.scalar_tensor_tensor(
                        out=o_t[:, k * T : (k + 1) * T],
                        in0=n_t[:, k * T : (k + 1) * T],
                        scalar=s_t[:, :],
                        in1=ps[:],
                        op0=mybir.AluOpType.mult,
                        op1=mybir.AluOpType.add,
                    )
                nc.sync.dma_start(o_d[:, :, st, :], o_t[:])

    nc.compile()
    return nc


def _get_program():
    if "nc" not in _CACHE:
        _CACHE["nc"] = _build_program()
    return _CACHE["nc"]


def _host_scalars(W, H, P, stddev):
    """M' (4x4 mix), s' (noise scale) -> S_mat [128,128], s_pp [128,1] f32."""
    W64 = np.asarray(W, np.float64)
    H64 = np.asarray(H, np.float64)
    P64 = np.asarray(P, np.float64)
    sd64 = np.asarray(stddev, np.float64)
    sqrtP = np.sqrt(P64)
    A = H64.T @ (W64 * sqrtP[None, :])  # A[u,v] = sum_n H[n,u] W[n,v] sqrtP[v]
    amp = np.diag(A).copy()
    Mp = A / amp[:, None]
    sp = sd64 / amp
    S_mat = np.kron(Mp.T, np.eye(Q, dtype=np.float64)).astype(np.float32)
    s_pp = np.repeat(sp, Q).astype(np.float32).reshape(128, 1)
    return np.ascontiguousarray(S_mat), s_pp


def make_in_maps(x, W, H, P, stddev, noise):
    S_mat, s_pp = _host_scalars(W, H, P, stddev)
    x = np.asarray(x, np.float32)
    noise = np.asarray(noise, np.float32)
    in_maps = []
    for c in range(NCORES):
        xs = np.ascontiguousarray(x[:, c * BL : (c + 1) * BL, :]).reshape(
            U, Q, NSUP, F
        )
        ns = np.ascontiguousarray(noise[:, c * BL : (c + 1) * BL, :]).reshape(
            U, Q, NSUP, F
        )
        in_maps.append({"x_s": xs, "n_s": ns, "S_mat": S_mat, "s_pp": s_pp})
    return in_maps


def gather_output(results):
    out = np.empty((U, BATCH, CWH), np.float32)
    for c in range(NCORES):
        out[:, c * BL : (c + 1) * BL, :] = results[c]["out_s"].reshape(U, BL, CWH)
    return out


def run_on_hw(x, W, H, P, stddev, noise, **run_kwargs):
    nc = _get_program()
    in_maps = make_in_maps(x, W, H, P, stddev, noise)
    res = bass_utils.run_bass_kernel_spmd(
        nc, in_maps, core_ids=list(range(NCORES)), **run_kwargs
    )
    return res


def kernel(x, W, H, P, stddev, noise):
    res = run_on_hw(x, W, H, P, stddev, noise)
    return gather_output(res.results)
